# revision 31
# baseline (speedup 1.0000x reference)
"""Trainium2 Bass kernel for DigitConvolutionalModel.

Model: x[B,784] -> reshape 28x28 -> 3x3 valid conv (weights conv_w) ->
[B,676] -> Linear(676,100)+relu -> Linear(100,10)+relu -> Linear(10,10).

The conv is linear, so it folds into the first Linear: W1f = C @ w1 where
C[784,676] is the conv unfold matrix. The whole model becomes a 3-layer MLP
784 -> 100 -> 10 -> 10 with relu between layers.

Sharding: pure data parallel, batch split across 8 cores (8192 rows each).

Precision: x is cast host-side to fp8 e3m4 (rel err 0.0142 vs the 2e-2
gate, measured end-to-end on the fixed seed) — this halves HBM traffic vs
bf16 so the DMA stream stays well ahead of the PE, making the PE the sole
bottleneck. Weights stay bf16 (mixed bf16-stationary x fp8-moving matmul
verified exact on HW); PSUM accumulates fp32; h1/h2 bf16.

On-chip layout: activations stay feature-major ([features, batch] on SBUF
partitions) end to end:
    h1T[100,n] = sum_k W1c[k].T @ xT[k,n]        (7 chunks of 128)
    h2T[10,n]  = w2.T @ relu(h1T+b1)
    yT[10,n]   = w3.T @ relu(h2T+b2) + b3
Features are zero-padded 784->896 = 7*128 host-side, so L1 is exactly 7
full-width passes (no ragged 16-feature tail pass).

DMA strategy (measured): SDMA engines round-robin between queues at
packet granularity, so ANY concurrent transfer delays the
pipeline-critical one, and per-DMA completion receipts cost ~1.5-2us.
Therefore: ONE combined 644KB front DMA (all weights + biases + both
pair-0 supertiles — a single receipt gates the first matmul), then one
917KB DMA per pair, all on the sync queue, explicitly dep-chained so
the Tile scheduler cannot reorder them (observed: it ignores priority
for DMA ordering and will put pair-1 ahead of pair-0's second half).
Big transfers also run at ~341+ GB/s vs ~250 for 229KB ones.

L2/L3 slot: w2's outputs go to PSUM partitions 0-9 and w3's to 32-41 of
ONE 2-bank PSUM tile per pair, so the per-pair h2-relu and y-bias-add
each fuse into a single DVE tensor_scalar op (a [10,512] DVE op is ~97%
fixed overhead — op count, not element count, is what matters).

Batch is cut into 18 supertiles (9 pairs): [256,256] + [512]*14 +
[384,128]. The small first pair starts the PE during the cold HAM
window (small N costs nothing extra there); the tiny final supertile
shortens the serial L2/L3/store drain chain; middle tiles are 512
because warm matmul cost has a ~(398+N)/2.4 ns isolated floor. Within a
pair both supertiles share every LDWEIGHTS via ldweights=False.
"""

import numpy as np
import ml_dtypes

import concourse.bacc as bacc
import concourse.tile as tile
from concourse.tile import add_dep_helper
from concourse import mybir
from concourse.bass_utils import run_bass_kernel_spmd

N_CORES = 8
B = 65536
BC = B // N_CORES          # 8192 rows per core
NF = 784
NKC = 7                    # 128-feature chunks after padding to 896
NFP = NKC * 128            # 896 padded features
H1 = 100
HO = 10
F32 = mybir.dt.float32
BF16 = mybir.dt.bfloat16
F8E3 = mybir.dt.float8e3
NP_BF16 = ml_dtypes.bfloat16
NP_F8E3 = ml_dtypes.float8_e3m4

SIZES = [256, 256] + [512] * 14 + [384, 128]
assert sum(SIZES) == BC
OFFS = np.cumsum([0] + SIZES).tolist()
NPAIR = len(SIZES) // 2
TNMAX = 512
WARMUP_MMS = 8

# front blob byte layout (per partition): w1 chunks | w2/w3/biases | pair-0 x
_O_W1 = 0                          # [128, 700] bf16 = 1400 B
_O_W2 = 1400                       # [100, 10] bf16
_O_W3 = 1420                       # [10, 10] bf16
_O_B1 = 1440                       # [100, 1] f32
_O_B2 = 1444                       # [10, 1] f32
_O_B3 = 1448                       # [10, 1] f32 at partitions 32-41
_O_X0A = 1452                      # [128, 7*256] fp8
FBW = _O_X0A + NKC * SIZES[0]      # 3244 B/partition

# xt2: supertile 0b first (the front blob only carries weights + 0a so
# the first matmul's gating transfer is minimal), then pairs 1.. as
# contiguous per-pair blocks [a: 7*na | b: 7*nb]; _XOFF[p] is pair p's
# byte-column offset (index 0 unused)
X0BW = NKC * SIZES[1]
_XOFF = [0, X0BW]
for _p in range(1, NPAIR - 1):
    _XOFF.append(_XOFF[-1] + NKC * (SIZES[2 * _p] + SIZES[2 * _p + 1]))
XT2W = _XOFF[-1] + NKC * (SIZES[-2] + SIZES[-1])


def _build_nc():
    nc = bacc.Bacc(None, target_bir_lowering=False)

    front = nc.dram_tensor("front", [128, FBW], F8E3, kind="ExternalInput")
    xt2 = nc.dram_tensor("xt2", [128, XT2W], F8E3, kind="ExternalInput")
    yt = nc.dram_tensor("yt", [HO, BC], F32, kind="ExternalOutput")

    relu = mybir.ActivationFunctionType.Relu

    with tile.TileContext(nc) as tc:
        with (
            tc.tile_pool(name="const", bufs=1) as cpool,
            tc.tile_pool(name="io", bufs=3) as iopool,
            tc.tile_pool(name="act", bufs=4) as apool,
            tc.tile_pool(name="ps1", bufs=4, space="PSUM") as ps1,
            tc.tile_pool(name="ps2", bufs=2, space="PSUM") as ps2,
        ):
            front_s = cpool.tile([128, FBW], F8E3, tag="front")
            prev_load = [nc.sync.dma_start(front_s[:], front[:])]
            xb0_s = cpool.tile([128, X0BW], F8E3, tag="xb0")
            d0 = nc.sync.dma_start(xb0_s[:], xt2[:, 0:X0BW])
            add_dep_helper(d0.ins, prev_load[0].ins, sync=False,
                           reason="load stream order")
            prev_load[0] = d0

            def load_pair(p):
                na, nb = SIZES[2 * p], SIZES[2 * p + 1]
                w = NKC * (na + nb)
                xp = iopool.tile([128, w], F8E3, tag=f"xp{w}",
                                 bufs=(3 if w > 4000 else 1))
                d = nc.sync.dma_start(
                    xp[:], xt2[:, _XOFF[p]:_XOFF[p] + w])
                # keep the load stream in pair order; the scheduler
                # otherwise reorders DMAs and later pairs steal SDMA
                # bandwidth from the one the PE is waiting on
                add_dep_helper(d.ins, prev_load[0].ins, sync=False,
                               reason="load stream order")
                prev_load[0] = d
                return xp

            def w1_ap(k):
                return front_s[:, 2 * k * H1:2 * (k + 1) * H1].bitcast(BF16)

            w2_ap = front_s[0:H1, _O_W2:_O_W2 + 2 * HO].bitcast(BF16)
            w3_ap = front_s[0:HO, _O_W3:_O_W3 + 2 * HO].bitcast(BF16)
            b1_ap = front_s[0:H1, _O_B1:_O_B1 + 4].bitcast(F32)
            b2_ap = front_s[0:HO, _O_B2:_O_B2 + 4].bitcast(F32)
            # b3 lives at partitions 32-41, lane-aligned with the L3 PSUM
            # outputs it is added to
            b3_32_ap = front_s[32:42, _O_B3:_O_B3 + 4].bitcast(F32)

            # All matmuls chained with same-engine ordering deps so the PE
            # executes them in emission order — required for ldweights=False
            prev_mm = [None]

            def mm(out_ap, lhsT_ap, rhs_ap, start, stop, ldw=True):
                m = nc.tensor.matmul(out_ap, lhsT_ap, rhs_ap,
                                     start=start, stop=stop)
                if not ldw:
                    m.ins.ldweights = False
                if prev_mm[0] is not None:
                    add_dep_helper(m.ins, prev_mm[0], sync=False,
                                   reason="pe program order")
                prev_mm[0] = m.ins
                return m

            # Warmup: dummy matmuls fill the NEFF startup ramp so the PE's
            # HAM throttle reaches full clock (~3.4us sustained) right as
            # the front blob's completion releases the first real matmul
            wsc = cpool.tile([128, TNMAX], BF16, tag="wsc")
            wp0 = ps1.tile([H1, TNMAX], F32, tag="p1")
            wp1 = ps1.tile([H1, TNMAX], F32, tag="p1")
            wp = [wp0, wp1]
            wfirst = nc.tensor.matmul(wp[0][:], wsc[:, 0:H1], wsc[:],
                                      start=True, stop=True)
            for i in range(1, WARMUP_MMS):
                w_mm = nc.tensor.matmul(wp[i % 2][:], wsc[:, 0:H1], wsc[:],
                                        start=True, stop=True)
                w_mm.ins.ldweights = False
                add_dep_helper(w_mm.ins, wfirst.ins, sync=False,
                               reason="warmup weight reuse")
            # written AFTER the warmup reads (WAR, not RAW): the warmup
            # multiplies garbage on purpose, so it can start at the
            # engines-go barrier instead of waiting for the memset
            nc.vector.memset(wsc[:], 0.0)

            # Pipeline over supertile pairs: at step p emit L1(p), then one
            # PE slot with L2(p-1) into PSUM parts 0-9 and L3(p-2) into
            # parts 32-41 of the pair's shared 2-bank tile.
            h1s: dict[int, object] = {}
            h2s: dict[int, object] = {}
            p23s: dict[int, object] = {}
            for p in range(NPAIR + 2):
                if p < NPAIR:
                    s0 = 2 * p
                    na, nb = SIZES[s0], SIZES[s0 + 1]
                    if p == 0:
                        def xa_k(k, _na=na):
                            return front_s[:, _O_X0A + k * _na:
                                           _O_X0A + (k + 1) * _na]

                        def xb_k(k, _nb=nb):
                            return xb0_s[:, k * _nb:(k + 1) * _nb]
                    else:
                        xp = load_pair(p)

                        def xa_k(k, _xp=xp, _na=na):
                            return _xp[:, k * _na:(k + 1) * _na]

                        def xb_k(k, _xp=xp, _na=na, _nb=nb):
                            o = NKC * _na
                            return _xp[:, o + k * _nb:o + (k + 1) * _nb]
                    p1a = ps1.tile([H1, TNMAX], F32, tag="p1")
                    p1b = ps1.tile([H1, TNMAX], F32, tag="p1")
                    if p == 0 or p == NPAIR - 1:
                        # first pair: all of a's chunks first so the PE
                        # starts as soon as the front blob lands; b
                        # reversed (reuses the just-loaded chunk-6
                        # weights). Last pair: a-side finishes early,
                        # shortening the L2/L3 drain chain.
                        for k in range(NKC):
                            mm(p1a[:, :na], w1_ap(k), xa_k(k),
                               start=(k == 0), stop=(k == NKC - 1))
                        for k in reversed(range(NKC)):
                            mm(p1b[:, :nb], w1_ap(k), xb_k(k),
                               start=(k == NKC - 1), stop=(k == 0),
                               ldw=(k != NKC - 1))
                    else:
                        for k in range(NKC):
                            mm(p1a[:, :na], w1_ap(k), xa_k(k),
                               start=(k == 0), stop=(k == NKC - 1))
                            mm(p1b[:, :nb], w1_ap(k), xb_k(k),
                               start=(k == 0), stop=(k == NKC - 1),
                               ldw=False)
                    for j, p1, n in ((0, p1a, na), (1, p1b, nb)):
                        h1 = apool.tile([H1, TNMAX], BF16, tag="h1")
                        if p == NPAIR - 1 and j == 1:
                            # last pair: this relu on DVE so both halves
                            # relu in parallel (tail latency)
                            nc.vector.scalar_tensor_tensor(
                                h1[:, :n], p1[:, :n], b1_ap,
                                wsc[0:H1, :n],
                                op0=mybir.AluOpType.add,
                                op1=mybir.AluOpType.max,
                            )
                        else:
                            nc.scalar.activation(h1[:, :n], p1[:, :n],
                                                 relu, bias=b1_ap)
                        h1s[s0 + j] = h1

                q = p - 1
                r = p - 2
                if 0 <= q < NPAIR:
                    nqa, nqb = SIZES[2 * q], SIZES[2 * q + 1]
                    # one 2-bank PSUM tile per pair: both halves of L2 and
                    # (next step) L3 land in it, so the DVE post-ops fuse
                    # into single instructions
                    tq = ps2.tile([42, 2, TNMAX], F32, tag="p23")
                    p23s[q] = tq
                    mm(tq[0:HO, 0, :nqa], w2_ap, h1s.pop(2 * q)[:, :nqa],
                       start=True, stop=True)
                    mm(tq[0:HO, 1, :nqb], w2_ap, h1s.pop(2 * q + 1)[:, :nqb],
                       start=True, stop=True, ldw=False)
                if 0 <= r < NPAIR:
                    nra, nrb = SIZES[2 * r], SIZES[2 * r + 1]
                    tr = p23s[r]
                    mm(tr[32:42, 0, :nra], w3_ap, h2s.pop(2 * r)[:, :nra],
                       start=True, stop=True)
                    mm(tr[32:42, 1, :nrb], w3_ap, h2s.pop(2 * r + 1)[:, :nrb],
                       start=True, stop=True, ldw=False)

                ot = None
                if 0 <= r < NPAIR:
                    nra, nrb = SIZES[2 * r], SIZES[2 * r + 1]
                    tr = p23s.pop(r)
                    # ot lives at SBUF partitions 32-41, lane-aligned with
                    # the PSUM partitions the L3 matmuls wrote (engines
                    # cannot shift partitions; the DMA reshapes for free).
                    # The y-add is emitted BEFORE this step's h2 op: the
                    # L2 matmuls of pair r+2 have a WAR dependency on this
                    # read (ps2 buffer reuse), so it must clear the DVE
                    # queue as early as possible
                    ot = apool.tile([42, 2, TNMAX], F32, tag="ot")
                    if nra == nrb:
                        nc.vector.tensor_scalar_add(
                            ot[32:42, :, :nra], tr[32:42, :, :nra], b3_32_ap)
                    else:
                        nc.vector.tensor_scalar_add(
                            ot[32:42, 0, :nra], tr[32:42, 0, :nra], b3_32_ap)
                        nc.vector.tensor_scalar_add(
                            ot[32:42, 1, :nrb], tr[32:42, 1, :nrb], b3_32_ap)

                if 0 <= q < NPAIR:
                    # h2 relu on DVE (tensor_scalar: +b2 then max 0) —
                    # keeps the ACT queue free for h1 relus; one fused op
                    # when the halves match. Not needed until the PE slot
                    # one step later, so it queues after the y-add.
                    h2t = apool.tile([HO, 2, TNMAX], BF16, tag="h2")
                    if nqa == nqb:
                        nc.vector.tensor_scalar(
                            h2t[:, :, :nqa], tq[0:HO, :, :nqa], b2_ap, 0.0,
                            op0=mybir.AluOpType.add,
                            op1=mybir.AluOpType.max,
                        )
                    else:
                        for j, n in ((0, nqa), (1, nqb)):
                            nc.vector.tensor_scalar(
                                h2t[:, j, :n], tq[0:HO, j, :n], b2_ap, 0.0,
                                op0=mybir.AluOpType.add,
                                op1=mybir.AluOpType.max,
                            )
                    h2s[2 * q] = h2t[:, 0]
                    h2s[2 * q + 1] = h2t[:, 1]

                if 0 <= r < NPAIR:
                    nra, nrb = SIZES[2 * r], SIZES[2 * r + 1]
                    c0 = OFFS[2 * r]
                    # gpsimd (SWDGE): stores must not sit in the sync
                    # (load) or scalar (ACT relu) in-order streams; the
                    # final pair stores each half separately on the
                    # by-then-idle HWDGE queues (lower completion latency,
                    # receipts overlap)
                    if r < NPAIR - 1:
                        nc.gpsimd.dma_start(yt[:, c0:c0 + 2 * nra],
                                            ot[32:42, :, :nra])
                    else:
                        nc.sync.dma_start(yt[:, c0:c0 + nra],
                                          ot[32:42, 0, :nra])
                        nc.scalar.dma_start(yt[:, c0 + nra:c0 + nra + nrb],
                                            ot[32:42, 1, :nrb])

    nc.compile()
    return nc


def _fold_conv_into_w1(conv_w: np.ndarray, w1: np.ndarray) -> np.ndarray:
    """W1f[784,100] such that x @ W1f == conv(x).reshape(B,676) @ w1."""
    c = np.zeros((NF, 26 * 26), dtype=np.float64)
    for di in range(3):
        for dj in range(3):
            ii, jj = np.meshgrid(np.arange(26), np.arange(26), indexing="ij")
            src = (ii + di) * 28 + (jj + dj)
            dst = ii * 26 + jj
            c[src.ravel(), dst.ravel()] += np.float64(conv_w[di, dj])
    return (c @ w1.astype(np.float64)).astype(np.float32)


def _x_block(xc8, lo, n):
    """[n rows, 896 feats] -> [128, 7*n] feature-major chunk layout."""
    blk = np.zeros((n, NFP), NP_F8E3)
    blk[:, :NF] = xc8[lo:lo + n]
    return blk.reshape(n, NKC, 128).transpose(2, 1, 0).reshape(128, NKC * n)


def _prep_in_maps(x, conv_w, w1, b1, w2, b2, w3, b3):
    x = np.asarray(x, dtype=np.float32)
    conv_w = np.asarray(conv_w, dtype=np.float32)
    w1 = np.asarray(w1, dtype=np.float32)
    b1 = np.asarray(b1, dtype=np.float32)
    w2 = np.asarray(w2, dtype=np.float32)
    b2 = np.asarray(b2, dtype=np.float32)
    w3 = np.asarray(w3, dtype=np.float32)
    b3 = np.asarray(b3, dtype=np.float32)

    w1f = _fold_conv_into_w1(conv_w, w1)  # [784, 100]
    w1p = np.zeros((NFP, H1), np.float32)
    w1p[:NF] = w1f
    # chunk-major: feature f = k*128 + p -> bytes 2*(k*100+m)
    w1m = np.ascontiguousarray(
        w1p.reshape(NKC, 128, H1).transpose(1, 0, 2)
    ).astype(NP_BF16).reshape(128, NKC * H1)

    wmix = np.zeros((128, FBW), np.uint8)
    wmix[:, _O_W1:_O_W1 + 2 * NKC * H1] = w1m.view(np.uint8)
    wmix[0:H1, _O_W2:_O_W2 + 2 * HO] = w2.astype(NP_BF16).view(np.uint8)
    wmix[0:HO, _O_W3:_O_W3 + 2 * HO] = w3.astype(NP_BF16).view(np.uint8)
    wmix[0:H1, _O_B1:_O_B1 + 4] = b1.reshape(H1, 1).view(np.uint8)
    wmix[0:HO, _O_B2:_O_B2 + 4] = b2.reshape(HO, 1).view(np.uint8)
    wmix[32:42, _O_B3:_O_B3 + 4] = b3.reshape(HO, 1).view(np.uint8)

    x8 = x.astype(NP_F8E3)  # cast once, full batch
    in_maps = []
    for core in range(N_CORES):
        xc8 = x8[core * BC:(core + 1) * BC]  # [8192, 784] fp8
        frontc = wmix.copy()
        frontc[:, _O_X0A:_O_X0A + NKC * SIZES[0]] = _x_block(
            xc8, OFFS[0], SIZES[0]).view(np.uint8)
        xt2c = np.empty((128, XT2W), NP_F8E3)
        xt2c[:, 0:X0BW] = _x_block(xc8, OFFS[1], SIZES[1])
        for p in range(1, NPAIR):
            na, nb = SIZES[2 * p], SIZES[2 * p + 1]
            o = _XOFF[p]
            xt2c[:, o:o + NKC * na] = _x_block(xc8, OFFS[2 * p], na)
            xt2c[:, o + NKC * na:o + NKC * (na + nb)] = _x_block(
                xc8, OFFS[2 * p + 1], nb)
        in_maps.append({"front": frontc.view(NP_F8E3), "xt2": xt2c})
    return in_maps


_NC = None


def _get_nc():
    global _NC
    if _NC is None:
        _NC = _build_nc()
    return _NC


def kernel(x, conv_w, w1, b1, w2, b2, w3, b3):
    in_maps = _prep_in_maps(x, conv_w, w1, b1, w2, b2, w3, b3)
    nc = _get_nc()
    res = run_bass_kernel_spmd(nc, in_maps, core_ids=list(range(N_CORES)))
    out = np.empty((B, HO), dtype=np.float32)
    for i in range(N_CORES):
        out[i * BC:(i + 1) * BC] = res.results[i]["yt"].T
    return out


if __name__ == "__main__":
    rng = np.random.default_rng(0)
    inputs = {
        "x": rng.standard_normal((B, NF), dtype=np.float32),
        "conv_w": np.ones((3, 3), dtype=np.float32),
        "w1": (rng.standard_normal((676, H1)) * 0.04).astype(np.float32),
        "b1": np.zeros(H1, dtype=np.float32),
        "w2": (rng.standard_normal((H1, HO)) * 0.1).astype(np.float32),
        "b2": np.zeros(HO, dtype=np.float32),
        "w3": (rng.standard_normal((HO, HO)) * 0.3).astype(np.float32),
        "b3": np.zeros(HO, dtype=np.float32),
    }
    out = kernel(**inputs)
    print(out.shape, out.dtype)


# revision 38
# speedup vs baseline: 1.0649x; 1.0649x over previous
"""Trainium2 Bass kernel for DigitConvolutionalModel.

Model: x[B,784] -> reshape 28x28 -> 3x3 valid conv (weights conv_w) ->
[B,676] -> Linear(676,100)+relu -> Linear(100,10)+relu -> Linear(10,10).

The conv is linear, so it folds into the first Linear: W1f = C @ w1 where
C[784,676] is the conv unfold matrix. The whole model becomes a 3-layer MLP
784 -> 100 -> 10 -> 10 with relu between layers.

Sharding: pure data parallel, batch split across 8 cores (8192 rows each).

Precision: x is cast host-side to fp8 e3m4 (rel err 0.0142 vs the 2e-2
gate, measured end-to-end on the fixed seed) — this halves HBM traffic vs
bf16 so the DMA stream stays well ahead of the PE, making the PE the sole
bottleneck. Weights stay bf16 (mixed bf16-stationary x fp8-moving matmul
verified exact on HW); PSUM accumulates fp32; h1/h2 bf16.

On-chip layout: activations stay feature-major ([features, batch] on SBUF
partitions) end to end:
    h1T[100,n] = sum_k W1c[k].T @ xT[k,n]        (7 chunks of 128)
    h2T[10,n]  = w2.T @ relu(h1T+b1)
    yT[10,n]   = w3.T @ relu(h2T+b2) + b3
Features are zero-padded 784->896 = 7*128 host-side, so L1 is exactly 7
full-width passes (no ragged 16-feature tail pass).

DMA strategy (measured): SDMA engines round-robin between queues at
packet granularity, so ANY concurrent transfer delays the
pipeline-critical one, and per-DMA completion receipts cost ~1.5-2us.
Therefore: ONE combined 644KB front DMA (all weights + biases + both
pair-0 supertiles — a single receipt gates the first matmul), then one
917KB DMA per pair, all on the sync queue, explicitly dep-chained so
the Tile scheduler cannot reorder them (observed: it ignores priority
for DMA ordering and will put pair-1 ahead of pair-0's second half).
Big transfers also run at ~341+ GB/s vs ~250 for 229KB ones.

L2/L3 slot: w2's outputs go to PSUM partitions 0-9 and w3's to 32-41 of
ONE 2-bank PSUM tile per pair, so the per-pair h2-relu and y-bias-add
each fuse into a single DVE tensor_scalar op (a [10,512] DVE op is ~97%
fixed overhead — op count, not element count, is what matters).

Batch is cut into 18 supertiles (9 pairs): [256,256] + [512]*14 +
[384,128]. The small first pair starts the PE during the cold HAM
window (small N costs nothing extra there); the tiny final supertile
shortens the serial L2/L3/store drain chain; middle tiles are 512
because warm matmul cost has a ~(398+N)/2.4 ns isolated floor. Within a
pair both supertiles share every LDWEIGHTS via ldweights=False.
"""

import numpy as np
import ml_dtypes

import concourse.bacc as bacc
import concourse.tile as tile
from concourse.tile import add_dep_helper
from concourse import mybir
from concourse.bass_utils import run_bass_kernel_spmd

N_CORES = 8
B = 65536
BC = B // N_CORES          # 8192 rows per core
NF = 784
NKC = 7                    # 128-feature chunks after padding to 896
NFP = NKC * 128            # 896 padded features
H1 = 100
HO = 10
F32 = mybir.dt.float32
BF16 = mybir.dt.bfloat16
F8E3 = mybir.dt.float8e3
NP_BF16 = ml_dtypes.bfloat16
NP_F8E3 = ml_dtypes.float8_e3m4

SIZES = [256, 256, 384, 512] + [448, 448] * 7 + [384, 128]
assert sum(SIZES) == BC
OFFS = np.cumsum([0] + SIZES).tolist()
NPAIR = len(SIZES) // 2
TNMAX = 512
WARMUP_MMS = 8

# front blob byte layout (per partition): w1 chunks | w2/w3/biases | pair-0 x
_O_W1 = 0                          # [128, 700] bf16 = 1400 B
_O_W2 = 1400                       # [100, 10] bf16
_O_W3 = 1420                       # [10, 10] bf16
_O_B1 = 1440                       # [100, 1] f32
_O_B2 = 1444                       # [10, 1] f32
_O_B3 = 1448                       # [10, 1] f32 at partitions 32-41
_O_X0A = 1452                      # [128, 7*256] fp8
FBW = _O_X0A + NKC * SIZES[0]      # 3244 B/partition

# xt2: per-supertile blocks for supertiles 1.. (supertile 0 rides in the
# front blob); loaded one DMA per supertile, strictly chained, so each
# pair's a-side lands well before its b-side is needed and a late
# transfer can never idle the PE past the ~3.4us HAM re-throttle window
_SOFF = [0] * len(SIZES)
_acc = 0
for _s in range(1, len(SIZES)):
    _SOFF[_s] = _acc
    _acc += NKC * SIZES[_s]
XT2W = _acc


def _build_nc():
    nc = bacc.Bacc(None, target_bir_lowering=False)

    front = nc.dram_tensor("front", [128, FBW], F8E3, kind="ExternalInput")
    xt2 = nc.dram_tensor("xt2", [128, XT2W], F8E3, kind="ExternalInput")
    yt = nc.dram_tensor("yt", [HO, BC], F32, kind="ExternalOutput")

    relu = mybir.ActivationFunctionType.Relu

    with tile.TileContext(nc) as tc:
        with (
            tc.tile_pool(name="const", bufs=1) as cpool,
            tc.tile_pool(name="io", bufs=3) as iopool,
            tc.tile_pool(name="act", bufs=4) as apool,
            tc.tile_pool(name="ps1", bufs=4, space="PSUM") as ps1,
            tc.tile_pool(name="ps2", bufs=2, space="PSUM") as ps2,
        ):
            front_s = cpool.tile([128, FBW], F8E3, tag="front")
            prev_load = [nc.sync.dma_start(front_s[:], front[:])]

            def load_sup(s):
                n = SIZES[s]
                w = NKC * n
                xm = iopool.tile([128, w], F8E3, tag=f"xm{n}",
                                 bufs=(6 if n >= 448 else 4))
                d = nc.sync.dma_start(xm[:], xt2[:, _SOFF[s]:_SOFF[s] + w])
                # keep the load stream in supertile order; the scheduler
                # otherwise reorders DMAs and later transfers steal SDMA
                # bandwidth from the one the PE is waiting on
                add_dep_helper(d.ins, prev_load[0].ins, sync=False,
                               reason="load stream order")
                prev_load[0] = d
                return xm

            def w1_ap(k):
                return front_s[:, 2 * k * H1:2 * (k + 1) * H1].bitcast(BF16)

            w2_ap = front_s[0:H1, _O_W2:_O_W2 + 2 * HO].bitcast(BF16)
            w3_ap = front_s[0:HO, _O_W3:_O_W3 + 2 * HO].bitcast(BF16)
            b1_ap = front_s[0:H1, _O_B1:_O_B1 + 4].bitcast(F32)
            b2_ap = front_s[0:HO, _O_B2:_O_B2 + 4].bitcast(F32)
            # b3 lives at partitions 32-41, lane-aligned with the L3 PSUM
            # outputs it is added to
            b3_32_ap = front_s[32:42, _O_B3:_O_B3 + 4].bitcast(F32)

            # All matmuls chained with same-engine ordering deps so the PE
            # executes them in emission order — required for ldweights=False
            prev_mm = [None]

            def mm(out_ap, lhsT_ap, rhs_ap, start, stop, ldw=True):
                m = nc.tensor.matmul(out_ap, lhsT_ap, rhs_ap,
                                     start=start, stop=stop)
                if not ldw:
                    m.ins.ldweights = False
                if prev_mm[0] is not None:
                    add_dep_helper(m.ins, prev_mm[0], sync=False,
                                   reason="pe program order")
                prev_mm[0] = m.ins
                return m

            # Warmup: dummy matmuls fill the NEFF startup ramp so the PE's
            # HAM throttle reaches full clock (~3.4us sustained) right as
            # the front blob's completion releases the first real matmul
            wsc = cpool.tile([128, TNMAX], BF16, tag="wsc")
            wp0 = ps1.tile([H1, TNMAX], F32, tag="p1")
            wp1 = ps1.tile([H1, TNMAX], F32, tag="p1")
            wp = [wp0, wp1]
            wfirst = nc.tensor.matmul(wp[0][:], wsc[:, 0:H1], wsc[:],
                                      start=True, stop=True)
            for i in range(1, WARMUP_MMS):
                w_mm = nc.tensor.matmul(wp[i % 2][:], wsc[:, 0:H1], wsc[:],
                                        start=True, stop=True)
                w_mm.ins.ldweights = False
                add_dep_helper(w_mm.ins, wfirst.ins, sync=False,
                               reason="warmup weight reuse")
            # written AFTER the warmup reads (WAR, not RAW): the warmup
            # multiplies garbage on purpose, so it can start at the
            # engines-go barrier instead of waiting for the memset
            nc.vector.memset(wsc[:], 0.0)

            # Pipeline over supertile pairs: at step p emit L1(p), then one
            # PE slot with L2(p-1) into PSUM parts 0-9 and L3(p-2) into
            # parts 32-41 of the pair's shared 2-bank tile.
            h1s: dict[int, object] = {}
            h2s: dict[int, object] = {}
            p23s: dict[int, object] = {}
            for p in range(NPAIR + 2):
                if p < NPAIR:
                    s0 = 2 * p
                    na, nb = SIZES[s0], SIZES[s0 + 1]
                    if p == 0:
                        def xa_k(k, _na=na):
                            return front_s[:, _O_X0A + k * _na:
                                           _O_X0A + (k + 1) * _na]
                        xb = load_sup(1)
                    else:
                        xa = load_sup(2 * p)
                        xb = load_sup(2 * p + 1)

                        def xa_k(k, _xa=xa, _na=na):
                            return _xa[:, k * _na:(k + 1) * _na]

                    def xb_k(k, _xb=xb, _nb=nb):
                        return _xb[:, k * _nb:(k + 1) * _nb]
                    p1a = ps1.tile([H1, TNMAX], F32, tag="p1")
                    p1b = ps1.tile([H1, TNMAX], F32, tag="p1")
                    # all of a's chunks first (the pair can start before
                    # its b-side load lands), then b reversed so the first
                    # b matmul reuses the just-loaded chunk-6 weights
                    for k in range(NKC):
                        mm(p1a[:, :na], w1_ap(k), xa_k(k),
                           start=(k == 0), stop=(k == NKC - 1))
                    for k in reversed(range(NKC)):
                        mm(p1b[:, :nb], w1_ap(k), xb_k(k),
                           start=(k == NKC - 1), stop=(k == 0),
                           ldw=(k != NKC - 1))
                    for j, p1, n in ((0, p1a, na), (1, p1b, nb)):
                        h1 = apool.tile([H1, TNMAX], BF16, tag="h1")
                        if p == NPAIR - 1 and j == 1:
                            # last pair: this relu on DVE so both halves
                            # relu in parallel (tail latency)
                            nc.vector.scalar_tensor_tensor(
                                h1[:, :n], p1[:, :n], b1_ap,
                                wsc[0:H1, :n],
                                op0=mybir.AluOpType.add,
                                op1=mybir.AluOpType.max,
                            )
                        else:
                            nc.scalar.activation(h1[:, :n], p1[:, :n],
                                                 relu, bias=b1_ap)
                        h1s[s0 + j] = h1

                q = p - 1
                r = p - 2
                if 0 <= q < NPAIR:
                    nqa, nqb = SIZES[2 * q], SIZES[2 * q + 1]
                    # one 2-bank PSUM tile per pair: both halves of L2 and
                    # (next step) L3 land in it, so the DVE post-ops fuse
                    # into single instructions
                    tq = ps2.tile([42, 2, TNMAX], F32, tag="p23")
                    p23s[q] = tq
                    mm(tq[0:HO, 0, :nqa], w2_ap, h1s.pop(2 * q)[:, :nqa],
                       start=True, stop=True)
                    mm(tq[0:HO, 1, :nqb], w2_ap, h1s.pop(2 * q + 1)[:, :nqb],
                       start=True, stop=True, ldw=False)
                if 0 <= r < NPAIR:
                    nra, nrb = SIZES[2 * r], SIZES[2 * r + 1]
                    tr = p23s[r]
                    mm(tr[32:42, 0, :nra], w3_ap, h2s.pop(2 * r)[:, :nra],
                       start=True, stop=True)
                    mm(tr[32:42, 1, :nrb], w3_ap, h2s.pop(2 * r + 1)[:, :nrb],
                       start=True, stop=True, ldw=False)

                def emit_y():
                    # ot lives at SBUF partitions 32-41, lane-aligned with
                    # the PSUM partitions the L3 matmuls wrote (engines
                    # cannot shift partitions; the DMA reshapes for free)
                    nra, nrb = SIZES[2 * r], SIZES[2 * r + 1]
                    tr = p23s.pop(r)
                    ot = apool.tile([42, 2, TNMAX], F32, tag="ot",
                                    name=f"ot{r}")
                    if nra == nrb:
                        nc.vector.tensor_scalar_add(
                            ot[32:42, :, :nra], tr[32:42, :, :nra], b3_32_ap)
                    else:
                        nc.vector.tensor_scalar_add(
                            ot[32:42, 0, :nra], tr[32:42, 0, :nra], b3_32_ap)
                        nc.vector.tensor_scalar_add(
                            ot[32:42, 1, :nrb], tr[32:42, 1, :nrb], b3_32_ap)
                    return ot

                def emit_h2():
                    # h2 relu on DVE (tensor_scalar: +b2 then max 0) —
                    # keeps the ACT queue free for h1 relus; one fused op
                    # when the halves match
                    h2t = apool.tile([HO, 2, TNMAX], BF16, tag="h2",
                                     name=f"h2t{q}")
                    if nqa == nqb:
                        nc.vector.tensor_scalar(
                            h2t[:, :, :nqa], tq[0:HO, :, :nqa], b2_ap, 0.0,
                            op0=mybir.AluOpType.add,
                            op1=mybir.AluOpType.max,
                        )
                    else:
                        for j, n in ((0, nqa), (1, nqb)):
                            nc.vector.tensor_scalar(
                                h2t[:, j, :n], tq[0:HO, j, :n], b2_ap, 0.0,
                                op0=mybir.AluOpType.add,
                                op1=mybir.AluOpType.max,
                            )
                    h2s[2 * q] = h2t[:, 0]
                    h2s[2 * q + 1] = h2t[:, 1]

                # DVE ordering: mid-stream the y-add goes first — the L2
                # matmuls of pair r+2 have a WAR dependency on its PSUM
                # read (ps2 buffer reuse). In the drain (last two pairs)
                # h2 goes first instead: L3(q) is waiting on it and the
                # WAR no longer binds.
                ot = None
                if 0 <= q < NPAIR and q >= NPAIR - 2:
                    emit_h2()
                    if 0 <= r < NPAIR:
                        ot = emit_y()
                else:
                    if 0 <= r < NPAIR:
                        ot = emit_y()
                    if 0 <= q < NPAIR:
                        emit_h2()

                if 0 <= r < NPAIR:
                    nra, nrb = SIZES[2 * r], SIZES[2 * r + 1]
                    c0 = OFFS[2 * r]
                    # gpsimd (SWDGE): stores must not sit in the sync
                    # (load) or scalar (ACT relu) in-order streams; the
                    # final pair stores each half separately on the
                    # by-then-idle HWDGE queues (lower completion latency,
                    # receipts overlap)
                    if r < NPAIR - 1:
                        if nra == nrb:
                            nc.gpsimd.dma_start(yt[:, c0:c0 + 2 * nra],
                                                ot[32:42, :, :nra])
                        else:
                            nc.gpsimd.dma_start(yt[:, c0:c0 + nra],
                                                ot[32:42, 0, :nra])
                            nc.gpsimd.dma_start(
                                yt[:, c0 + nra:c0 + nra + nrb],
                                ot[32:42, 1, :nrb])
                    else:
                        nc.sync.dma_start(yt[:, c0:c0 + nra],
                                          ot[32:42, 0, :nra])
                        nc.scalar.dma_start(yt[:, c0 + nra:c0 + nra + nrb],
                                            ot[32:42, 1, :nrb])

    nc.compile()
    return nc


def _fold_conv_into_w1(conv_w: np.ndarray, w1: np.ndarray) -> np.ndarray:
    """W1f[784,100] such that x @ W1f == conv(x).reshape(B,676) @ w1."""
    c = np.zeros((NF, 26 * 26), dtype=np.float64)
    for di in range(3):
        for dj in range(3):
            ii, jj = np.meshgrid(np.arange(26), np.arange(26), indexing="ij")
            src = (ii + di) * 28 + (jj + dj)
            dst = ii * 26 + jj
            c[src.ravel(), dst.ravel()] += np.float64(conv_w[di, dj])
    return (c @ w1.astype(np.float64)).astype(np.float32)


def _x_block(xc8, lo, n):
    """[n rows, 896 feats] -> [128, 7*n] feature-major chunk layout."""
    blk = np.zeros((n, NFP), NP_F8E3)
    blk[:, :NF] = xc8[lo:lo + n]
    return blk.reshape(n, NKC, 128).transpose(2, 1, 0).reshape(128, NKC * n)


def _prep_in_maps(x, conv_w, w1, b1, w2, b2, w3, b3):
    x = np.asarray(x, dtype=np.float32)
    conv_w = np.asarray(conv_w, dtype=np.float32)
    w1 = np.asarray(w1, dtype=np.float32)
    b1 = np.asarray(b1, dtype=np.float32)
    w2 = np.asarray(w2, dtype=np.float32)
    b2 = np.asarray(b2, dtype=np.float32)
    w3 = np.asarray(w3, dtype=np.float32)
    b3 = np.asarray(b3, dtype=np.float32)

    w1f = _fold_conv_into_w1(conv_w, w1)  # [784, 100]
    w1p = np.zeros((NFP, H1), np.float32)
    w1p[:NF] = w1f
    # chunk-major: feature f = k*128 + p -> bytes 2*(k*100+m)
    w1m = np.ascontiguousarray(
        w1p.reshape(NKC, 128, H1).transpose(1, 0, 2)
    ).astype(NP_BF16).reshape(128, NKC * H1)

    wmix = np.zeros((128, FBW), np.uint8)
    wmix[:, _O_W1:_O_W1 + 2 * NKC * H1] = w1m.view(np.uint8)
    wmix[0:H1, _O_W2:_O_W2 + 2 * HO] = w2.astype(NP_BF16).view(np.uint8)
    wmix[0:HO, _O_W3:_O_W3 + 2 * HO] = w3.astype(NP_BF16).view(np.uint8)
    wmix[0:H1, _O_B1:_O_B1 + 4] = b1.reshape(H1, 1).view(np.uint8)
    wmix[0:HO, _O_B2:_O_B2 + 4] = b2.reshape(HO, 1).view(np.uint8)
    wmix[32:42, _O_B3:_O_B3 + 4] = b3.reshape(HO, 1).view(np.uint8)

    x8 = x.astype(NP_F8E3)  # cast once, full batch
    in_maps = []
    for core in range(N_CORES):
        xc8 = x8[core * BC:(core + 1) * BC]  # [8192, 784] fp8
        frontc = wmix.copy()
        frontc[:, _O_X0A:_O_X0A + NKC * SIZES[0]] = _x_block(
            xc8, OFFS[0], SIZES[0]).view(np.uint8)
        xt2c = np.empty((128, XT2W), NP_F8E3)
        for s in range(1, len(SIZES)):
            xt2c[:, _SOFF[s]:_SOFF[s] + NKC * SIZES[s]] = _x_block(
                xc8, OFFS[s], SIZES[s])
        in_maps.append({"front": frontc.view(NP_F8E3), "xt2": xt2c})
    return in_maps


_NC = None


def _get_nc():
    global _NC
    if _NC is None:
        _NC = _build_nc()
    return _NC


def kernel(x, conv_w, w1, b1, w2, b2, w3, b3):
    in_maps = _prep_in_maps(x, conv_w, w1, b1, w2, b2, w3, b3)
    nc = _get_nc()
    res = run_bass_kernel_spmd(nc, in_maps, core_ids=list(range(N_CORES)))
    out = np.empty((B, HO), dtype=np.float32)
    for i in range(N_CORES):
        out[i * BC:(i + 1) * BC] = res.results[i]["yt"].T
    return out


if __name__ == "__main__":
    rng = np.random.default_rng(0)
    inputs = {
        "x": rng.standard_normal((B, NF), dtype=np.float32),
        "conv_w": np.ones((3, 3), dtype=np.float32),
        "w1": (rng.standard_normal((676, H1)) * 0.04).astype(np.float32),
        "b1": np.zeros(H1, dtype=np.float32),
        "w2": (rng.standard_normal((H1, HO)) * 0.1).astype(np.float32),
        "b2": np.zeros(HO, dtype=np.float32),
        "w3": (rng.standard_normal((HO, HO)) * 0.3).astype(np.float32),
        "b3": np.zeros(HO, dtype=np.float32),
    }
    out = kernel(**inputs)
    print(out.shape, out.dtype)


# revision 41
# speedup vs baseline: 1.1075x; 1.0400x over previous
"""Trainium2 Bass kernel for DigitConvolutionalModel.

Model: x[B,784] -> reshape 28x28 -> 3x3 valid conv (weights conv_w) ->
[B,676] -> Linear(676,100)+relu -> Linear(100,10)+relu -> Linear(10,10).

The conv is linear, so it folds into the first Linear: W1f = C @ w1 where
C[784,676] is the conv unfold matrix. The whole model becomes a 3-layer MLP
784 -> 100 -> 10 -> 10 with relu between layers.

Sharding: pure data parallel, batch split across 8 cores (8192 rows each).

Precision: x is cast host-side to fp8 e3m4 (rel err 0.0142 vs the 2e-2
gate, measured end-to-end on the fixed seed) — this halves HBM traffic vs
bf16 so the DMA stream stays well ahead of the PE, making the PE the sole
bottleneck. Weights stay bf16 (mixed bf16-stationary x fp8-moving matmul
verified exact on HW); PSUM accumulates fp32; h1/h2 bf16.

On-chip layout: activations stay feature-major ([features, batch] on SBUF
partitions) end to end:
    h1T[100,n] = sum_k W1c[k].T @ xT[k,n]        (7 chunks of 128)
    h2T[10,n]  = w2.T @ relu(h1T+b1)
    yT[10,n]   = w3.T @ relu(h2T+b2) + b3
Features are zero-padded 784->896 = 7*128 host-side, so L1 is exactly 7
full-width passes (no ragged 16-feature tail pass).

DMA strategy (measured): SDMA engines round-robin between queues at
packet granularity, so ANY concurrent transfer delays the
pipeline-critical one, and per-DMA completion receipts cost ~1.5-2us.
Therefore: ONE combined 644KB front DMA (all weights + biases + both
pair-0 supertiles — a single receipt gates the first matmul), then one
917KB DMA per pair, all on the sync queue, explicitly dep-chained so
the Tile scheduler cannot reorder them (observed: it ignores priority
for DMA ordering and will put pair-1 ahead of pair-0's second half).
Big transfers also run at ~341+ GB/s vs ~250 for 229KB ones.

L2/L3 slot: w2's outputs go to PSUM partitions 0-9 and w3's to 32-41 of
ONE 2-bank PSUM tile per pair, so the per-pair h2-relu and y-bias-add
each fuse into a single DVE tensor_scalar op (a [10,512] DVE op is ~97%
fixed overhead — op count, not element count, is what matters).

Batch is cut into 18 supertiles (9 pairs): [256,256] + [512]*14 +
[384,128]. The small first pair starts the PE during the cold HAM
window (small N costs nothing extra there); the tiny final supertile
shortens the serial L2/L3/store drain chain; middle tiles are 512
because warm matmul cost has a ~(398+N)/2.4 ns isolated floor. Within a
pair both supertiles share every LDWEIGHTS via ldweights=False.
"""

import numpy as np
import ml_dtypes

import concourse.bacc as bacc
import concourse.tile as tile
from concourse.tile import add_dep_helper
from concourse import mybir
from concourse.bass_utils import run_bass_kernel_spmd

N_CORES = 8
B = 65536
BC = B // N_CORES          # 8192 rows per core
NF = 784
NKC = 7                    # 128-feature chunks after padding to 896
NFP = NKC * 128            # 896 padded features
H1 = 100
HO = 10
F32 = mybir.dt.float32
BF16 = mybir.dt.bfloat16
F8E3 = mybir.dt.float8e3
NP_BF16 = ml_dtypes.bfloat16
NP_F8E3 = ml_dtypes.float8_e3m4

SIZES = [256, 256, 384, 512] + [448, 448] * 7 + [384, 128]
assert sum(SIZES) == BC
OFFS = np.cumsum([0] + SIZES).tolist()
NPAIR = len(SIZES) // 2
TNMAX = 512
WARMUP_MMS = 9

# front blob byte layout (per partition): w1 chunks | w2/w3/biases | pair-0 x
_O_W1 = 0                          # [128, 700] bf16 = 1400 B
_O_W2 = 1400                       # [100, 10] bf16
_O_W3 = 1420                       # [10, 10] bf16
_O_B1 = 1440                       # [100, 1] f32
_O_B2 = 1444                       # [10, 1] f32
_O_B3 = 1448                       # [10, 1] f32 at partitions 32-41
_O_X0A = 1452                      # [128, 7*256] fp8
FBW = _O_X0A + NKC * SIZES[0]      # 3244 B/partition

# xt2: per-supertile blocks for supertiles 1.. (supertile 0 rides in the
# front blob); loaded one DMA per supertile, strictly chained, so each
# pair's a-side lands well before its b-side is needed and a late
# transfer can never idle the PE past the ~3.4us HAM re-throttle window
_SOFF = [0] * len(SIZES)
_acc = 0
for _s in range(1, len(SIZES)):
    _SOFF[_s] = _acc
    _acc += NKC * SIZES[_s]
XT2W = _acc


def _build_nc():
    nc = bacc.Bacc(None, target_bir_lowering=False)

    front = nc.dram_tensor("front", [128, FBW], F8E3, kind="ExternalInput")
    xt2 = nc.dram_tensor("xt2", [128, XT2W], F8E3, kind="ExternalInput")
    yt = nc.dram_tensor("yt", [HO, BC], F32, kind="ExternalOutput")

    relu = mybir.ActivationFunctionType.Relu

    with tile.TileContext(nc) as tc:
        with (
            tc.tile_pool(name="const", bufs=1) as cpool,
            tc.tile_pool(name="io", bufs=3) as iopool,
            tc.tile_pool(name="act", bufs=4) as apool,
            tc.tile_pool(name="ps1", bufs=4, space="PSUM") as ps1,
            tc.tile_pool(name="ps2", bufs=2, space="PSUM") as ps2,
        ):
            front_s = cpool.tile([128, FBW], F8E3, tag="front")
            prev_load = [nc.sync.dma_start(front_s[:], front[:])]

            def load_sup(s):
                n = SIZES[s]
                w = NKC * n
                xm = iopool.tile([128, w], F8E3, tag=f"xm{n}",
                                 bufs=(6 if n >= 448 else 4))
                d = nc.sync.dma_start(xm[:], xt2[:, _SOFF[s]:_SOFF[s] + w])
                # keep the load stream in supertile order; the scheduler
                # otherwise reorders DMAs and later transfers steal SDMA
                # bandwidth from the one the PE is waiting on
                add_dep_helper(d.ins, prev_load[0].ins, sync=False,
                               reason="load stream order")
                prev_load[0] = d
                return xm

            def w1_ap(k):
                return front_s[:, 2 * k * H1:2 * (k + 1) * H1].bitcast(BF16)

            w2_ap = front_s[0:H1, _O_W2:_O_W2 + 2 * HO].bitcast(BF16)
            w3_ap = front_s[0:HO, _O_W3:_O_W3 + 2 * HO].bitcast(BF16)
            b1_ap = front_s[0:H1, _O_B1:_O_B1 + 4].bitcast(F32)
            b2_ap = front_s[0:HO, _O_B2:_O_B2 + 4].bitcast(F32)
            # b3 lives at partitions 32-41, lane-aligned with the L3 PSUM
            # outputs it is added to
            b3_32_ap = front_s[32:42, _O_B3:_O_B3 + 4].bitcast(F32)

            # All matmuls chained with same-engine ordering deps so the PE
            # executes them in emission order — required for ldweights=False
            prev_mm = [None]

            def mm(out_ap, lhsT_ap, rhs_ap, start, stop, ldw=True):
                m = nc.tensor.matmul(out_ap, lhsT_ap, rhs_ap,
                                     start=start, stop=stop)
                if not ldw:
                    m.ins.ldweights = False
                if prev_mm[0] is not None:
                    add_dep_helper(m.ins, prev_mm[0], sync=False,
                                   reason="pe program order")
                prev_mm[0] = m.ins
                return m

            # DVE ops likewise: the Tile scheduler otherwise reorders the
            # DVE queue and the drain-critical h2 relu lands behind
            # non-critical y-adds
            prev_dve = [None]

            def dve(m):
                if prev_dve[0] is not None:
                    add_dep_helper(m.ins, prev_dve[0], sync=False,
                                   reason="dve program order")
                prev_dve[0] = m.ins
                return m

            # Warmup: dummy matmuls fill the NEFF startup ramp so the PE's
            # HAM throttle reaches full clock (~3.4us sustained) right as
            # the front blob's completion releases the first real matmul
            wsc = cpool.tile([128, TNMAX], BF16, tag="wsc")
            wp0 = ps1.tile([H1, TNMAX], F32, tag="p1")
            wp1 = ps1.tile([H1, TNMAX], F32, tag="p1")
            wp = [wp0, wp1]
            wfirst = nc.tensor.matmul(wp[0][:], wsc[:, 0:H1], wsc[:],
                                      start=True, stop=True)
            for i in range(1, WARMUP_MMS):
                w_mm = nc.tensor.matmul(wp[i % 2][:], wsc[:, 0:H1], wsc[:],
                                        start=True, stop=True)
                w_mm.ins.ldweights = False
                add_dep_helper(w_mm.ins, wfirst.ins, sync=False,
                               reason="warmup weight reuse")
            # written AFTER the warmup reads (WAR, not RAW): the warmup
            # multiplies garbage on purpose, so it can start at the
            # engines-go barrier instead of waiting for the memset
            nc.vector.memset(wsc[:], 0.0)

            # Pipeline over supertile pairs: at step p emit L1(p), then one
            # PE slot with L2(p-1) into PSUM parts 0-9 and L3(p-2) into
            # parts 32-41 of the pair's shared 2-bank tile.
            h1s: dict[int, object] = {}
            h2s: dict[int, object] = {}
            p23s: dict[int, object] = {}
            for p in range(NPAIR + 2):
                if p < NPAIR:
                    s0 = 2 * p
                    na, nb = SIZES[s0], SIZES[s0 + 1]
                    if p == 0:
                        def xa_k(k, _na=na):
                            return front_s[:, _O_X0A + k * _na:
                                           _O_X0A + (k + 1) * _na]
                        xb = load_sup(1)
                    else:
                        xa = load_sup(2 * p)
                        xb = load_sup(2 * p + 1)

                        def xa_k(k, _xa=xa, _na=na):
                            return _xa[:, k * _na:(k + 1) * _na]

                    def xb_k(k, _xb=xb, _nb=nb):
                        return _xb[:, k * _nb:(k + 1) * _nb]
                    p1a = ps1.tile([H1, TNMAX], F32, tag="p1")
                    p1b = ps1.tile([H1, TNMAX], F32, tag="p1")
                    # all of a's chunks first (the pair can start before
                    # its b-side load lands), then b reversed so the first
                    # b matmul reuses the just-loaded chunk-6 weights
                    for k in range(NKC):
                        mm(p1a[:, :na], w1_ap(k), xa_k(k),
                           start=(k == 0), stop=(k == NKC - 1))
                    for k in reversed(range(NKC)):
                        mm(p1b[:, :nb], w1_ap(k), xb_k(k),
                           start=(k == NKC - 1), stop=(k == 0),
                           ldw=(k != NKC - 1))
                    for j, p1, n in ((0, p1a, na), (1, p1b, nb)):
                        h1 = apool.tile([H1, TNMAX], BF16, tag="h1")
                        if p == NPAIR - 1 and j == 1:
                            # last pair: this relu on DVE so both halves
                            # relu in parallel (tail latency)
                            dve(nc.vector.scalar_tensor_tensor(
                                h1[:, :n], p1[:, :n], b1_ap,
                                wsc[0:H1, :n],
                                op0=mybir.AluOpType.add,
                                op1=mybir.AluOpType.max,
                            ))
                        else:
                            nc.scalar.activation(h1[:, :n], p1[:, :n],
                                                 relu, bias=b1_ap)
                        h1s[s0 + j] = h1

                q = p - 1
                r = p - 2
                if 0 <= q < NPAIR:
                    nqa, nqb = SIZES[2 * q], SIZES[2 * q + 1]
                    # one 2-bank PSUM tile per pair: both halves of L2 and
                    # (next step) L3 land in it, so the DVE post-ops fuse
                    # into single instructions
                    tq = ps2.tile([42, 2, TNMAX], F32, tag="p23")
                    p23s[q] = tq
                    mm(tq[0:HO, 0, :nqa], w2_ap, h1s.pop(2 * q)[:, :nqa],
                       start=True, stop=True)
                    mm(tq[0:HO, 1, :nqb], w2_ap, h1s.pop(2 * q + 1)[:, :nqb],
                       start=True, stop=True, ldw=False)
                if 0 <= r < NPAIR:
                    nra, nrb = SIZES[2 * r], SIZES[2 * r + 1]
                    tr = p23s[r]
                    mm(tr[32:42, 0, :nra], w3_ap, h2s.pop(2 * r)[:, :nra],
                       start=True, stop=True)
                    mm(tr[32:42, 1, :nrb], w3_ap, h2s.pop(2 * r + 1)[:, :nrb],
                       start=True, stop=True, ldw=False)

                def emit_y():
                    # ot lives at SBUF partitions 32-41, lane-aligned with
                    # the PSUM partitions the L3 matmuls wrote (engines
                    # cannot shift partitions; the DMA reshapes for free)
                    nra, nrb = SIZES[2 * r], SIZES[2 * r + 1]
                    tr = p23s.pop(r)
                    ot = apool.tile([42, 2, TNMAX], F32, tag="ot",
                                    name=f"ot{r}")
                    if nra == nrb:
                        dve(nc.vector.tensor_scalar_add(
                            ot[32:42, :, :nra], tr[32:42, :, :nra],
                            b3_32_ap))
                    else:
                        dve(nc.vector.tensor_scalar_add(
                            ot[32:42, 0, :nra], tr[32:42, 0, :nra],
                            b3_32_ap))
                        dve(nc.vector.tensor_scalar_add(
                            ot[32:42, 1, :nrb], tr[32:42, 1, :nrb],
                            b3_32_ap))
                    return ot

                def emit_h2():
                    # h2 relu on DVE (tensor_scalar: +b2 then max 0) —
                    # keeps the ACT queue free for h1 relus; one fused op
                    # when the halves match
                    h2t = apool.tile([HO, 2, TNMAX], BF16, tag="h2",
                                     name=f"h2t{q}")
                    if nqa == nqb:
                        dve(nc.vector.tensor_scalar(
                            h2t[:, :, :nqa], tq[0:HO, :, :nqa], b2_ap, 0.0,
                            op0=mybir.AluOpType.add,
                            op1=mybir.AluOpType.max,
                        ))
                    else:
                        for j, n in ((0, nqa), (1, nqb)):
                            dve(nc.vector.tensor_scalar(
                                h2t[:, j, :n], tq[0:HO, j, :n], b2_ap, 0.0,
                                op0=mybir.AluOpType.add,
                                op1=mybir.AluOpType.max,
                            ))
                    h2s[2 * q] = h2t[:, 0]
                    h2s[2 * q + 1] = h2t[:, 1]

                # DVE ordering: mid-stream the y-add goes first — the L2
                # matmuls of pair r+2 have a WAR dependency on its PSUM
                # read (ps2 buffer reuse). In the drain (last two pairs)
                # h2 goes first instead: L3(q) is waiting on it and the
                # WAR no longer binds.
                ot = None
                if 0 <= q < NPAIR and q >= NPAIR - 2:
                    emit_h2()
                    if 0 <= r < NPAIR:
                        ot = emit_y()
                else:
                    if 0 <= r < NPAIR:
                        ot = emit_y()
                    if 0 <= q < NPAIR:
                        emit_h2()

                if 0 <= r < NPAIR:
                    nra, nrb = SIZES[2 * r], SIZES[2 * r + 1]
                    c0 = OFFS[2 * r]
                    # gpsimd (SWDGE): stores must not sit in the sync
                    # (load) or scalar (ACT relu) in-order streams; the
                    # final pair stores each half separately on the
                    # by-then-idle HWDGE queues (lower completion latency,
                    # receipts overlap)
                    if r < NPAIR - 1:
                        if nra == nrb:
                            nc.gpsimd.dma_start(yt[:, c0:c0 + 2 * nra],
                                                ot[32:42, :, :nra])
                        else:
                            nc.gpsimd.dma_start(yt[:, c0:c0 + nra],
                                                ot[32:42, 0, :nra])
                            nc.gpsimd.dma_start(
                                yt[:, c0 + nra:c0 + nra + nrb],
                                ot[32:42, 1, :nrb])
                    else:
                        nc.sync.dma_start(yt[:, c0:c0 + nra],
                                          ot[32:42, 0, :nra])
                        nc.scalar.dma_start(yt[:, c0 + nra:c0 + nra + nrb],
                                            ot[32:42, 1, :nrb])

    nc.compile()
    return nc


def _fold_conv_into_w1(conv_w: np.ndarray, w1: np.ndarray) -> np.ndarray:
    """W1f[784,100] such that x @ W1f == conv(x).reshape(B,676) @ w1."""
    c = np.zeros((NF, 26 * 26), dtype=np.float64)
    for di in range(3):
        for dj in range(3):
            ii, jj = np.meshgrid(np.arange(26), np.arange(26), indexing="ij")
            src = (ii + di) * 28 + (jj + dj)
            dst = ii * 26 + jj
            c[src.ravel(), dst.ravel()] += np.float64(conv_w[di, dj])
    return (c @ w1.astype(np.float64)).astype(np.float32)


def _x_block(xc8, lo, n):
    """[n rows, 896 feats] -> [128, 7*n] feature-major chunk layout."""
    blk = np.zeros((n, NFP), NP_F8E3)
    blk[:, :NF] = xc8[lo:lo + n]
    return blk.reshape(n, NKC, 128).transpose(2, 1, 0).reshape(128, NKC * n)


def _prep_in_maps(x, conv_w, w1, b1, w2, b2, w3, b3):
    x = np.asarray(x, dtype=np.float32)
    conv_w = np.asarray(conv_w, dtype=np.float32)
    w1 = np.asarray(w1, dtype=np.float32)
    b1 = np.asarray(b1, dtype=np.float32)
    w2 = np.asarray(w2, dtype=np.float32)
    b2 = np.asarray(b2, dtype=np.float32)
    w3 = np.asarray(w3, dtype=np.float32)
    b3 = np.asarray(b3, dtype=np.float32)

    w1f = _fold_conv_into_w1(conv_w, w1)  # [784, 100]
    w1p = np.zeros((NFP, H1), np.float32)
    w1p[:NF] = w1f
    # chunk-major: feature f = k*128 + p -> bytes 2*(k*100+m)
    w1m = np.ascontiguousarray(
        w1p.reshape(NKC, 128, H1).transpose(1, 0, 2)
    ).astype(NP_BF16).reshape(128, NKC * H1)

    wmix = np.zeros((128, FBW), np.uint8)
    wmix[:, _O_W1:_O_W1 + 2 * NKC * H1] = w1m.view(np.uint8)
    wmix[0:H1, _O_W2:_O_W2 + 2 * HO] = w2.astype(NP_BF16).view(np.uint8)
    wmix[0:HO, _O_W3:_O_W3 + 2 * HO] = w3.astype(NP_BF16).view(np.uint8)
    wmix[0:H1, _O_B1:_O_B1 + 4] = b1.reshape(H1, 1).view(np.uint8)
    wmix[0:HO, _O_B2:_O_B2 + 4] = b2.reshape(HO, 1).view(np.uint8)
    wmix[32:42, _O_B3:_O_B3 + 4] = b3.reshape(HO, 1).view(np.uint8)

    x8 = x.astype(NP_F8E3)  # cast once, full batch
    in_maps = []
    for core in range(N_CORES):
        xc8 = x8[core * BC:(core + 1) * BC]  # [8192, 784] fp8
        frontc = wmix.copy()
        frontc[:, _O_X0A:_O_X0A + NKC * SIZES[0]] = _x_block(
            xc8, OFFS[0], SIZES[0]).view(np.uint8)
        xt2c = np.empty((128, XT2W), NP_F8E3)
        for s in range(1, len(SIZES)):
            xt2c[:, _SOFF[s]:_SOFF[s] + NKC * SIZES[s]] = _x_block(
                xc8, OFFS[s], SIZES[s])
        in_maps.append({"front": frontc.view(NP_F8E3), "xt2": xt2c})
    return in_maps


_NC = None


def _get_nc():
    global _NC
    if _NC is None:
        _NC = _build_nc()
    return _NC


def kernel(x, conv_w, w1, b1, w2, b2, w3, b3):
    in_maps = _prep_in_maps(x, conv_w, w1, b1, w2, b2, w3, b3)
    nc = _get_nc()
    res = run_bass_kernel_spmd(nc, in_maps, core_ids=list(range(N_CORES)))
    out = np.empty((B, HO), dtype=np.float32)
    for i in range(N_CORES):
        out[i * BC:(i + 1) * BC] = res.results[i]["yt"].T
    return out


if __name__ == "__main__":
    rng = np.random.default_rng(0)
    inputs = {
        "x": rng.standard_normal((B, NF), dtype=np.float32),
        "conv_w": np.ones((3, 3), dtype=np.float32),
        "w1": (rng.standard_normal((676, H1)) * 0.04).astype(np.float32),
        "b1": np.zeros(H1, dtype=np.float32),
        "w2": (rng.standard_normal((H1, HO)) * 0.1).astype(np.float32),
        "b2": np.zeros(HO, dtype=np.float32),
        "w3": (rng.standard_normal((HO, HO)) * 0.3).astype(np.float32),
        "b3": np.zeros(HO, dtype=np.float32),
    }
    out = kernel(**inputs)
    print(out.shape, out.dtype)
